# revision 1
# baseline (speedup 1.0000x reference)
"""Multi-head self-attention (B=4, S=4096, D=128, H=4, no scaling, no mask)
on 8 Trainium2 NeuronCores.

Sharding: 16 (batch, head) pairs over 8 cores -> core c handles batch c//2,
heads 2*(c%2) and 2*(c%2)+1. No cross-core communication.

Per-core algorithm (flash-style, scores never touch DRAM):
  xT [D=128, S] in SBUF (f32r via DMA bitcast)
  qT_rep[h] [96, S]  = (Wq_h @ x.T) replicated 3x across partition thirds
                       (host replicates Wq columns; one matmul per 512-chunk)
  kT_pack[h][32r+e, g*128+p] = k[(3g+r)*128+p, e]  -- packed for 3-way
                       row-tiled matmuls (zero-padded accumulating projection)
  vhat [S-part, j*66+h*33+c] = v columns for head h plus a ones column
                       (ones column yields softmax denominators for free)
  per (head h, q-block i0 of 512):
    for each group g of 3 k-chunks (128 each): 3 concurrent row-tiled
      matmuls (tile_position=(32r,0)) -> scoresT psum [128, 3*512]
      ACT Exp psum -> pT sbuf (f32r)
      3 accumulating matmuls av[33,512] += vhat_j.T @ pT_j
    av[32,:] = sum_k exp(s) -> reciprocal -> gpsimd partition broadcast
    out[32,512] = av[0:32,:] * bcast -> DMA to OUT[h, :, i0*512:+512]
Host gathers OUT [2, 32, S] per core into the full (B, S, D) output.
"""

import sys

for _p in ("/opt/trn_rl_repo", "/root/.axon_site/_ro/trn_rl_repo"):
    if _p not in sys.path:
        sys.path.append(_p)

import numpy as np
from contextlib import ExitStack

import concourse.bass as bass
import concourse.bacc as bacc
import concourse.mybir as mybir
import concourse.tile as tile
from concourse import bass_utils

F32 = mybir.dt.float32
F32R = mybir.dt.float32r
AF = mybir.ActivationFunctionType
ALU = mybir.AluOpType

B, D, H, HD = 4, 128, 4, 32
NCORES = 8

_built = {}


def build_nc(S):
    """Build + compile the per-core program (identical across cores)."""
    NJ = S // 128  # number of 128-wide k-chunks
    NQB = S // 512  # number of 512-wide q-blocks
    groups = [list(range(i, min(i + 3, NJ))) for i in range(0, NJ, 3)]
    NG3 = sum(1 for g in groups if len(g) == 3)  # full groups in kt_pack
    tail = [j for g in groups[NG3:] for j in g]  # 0..2 trailing chunks

    nc = bacc.Bacc("TRN2", target_bir_lowering=False, debug=False)

    XT = nc.dram_tensor("XT", [128, S], F32, kind="ExternalInput").ap()
    WBLOB = nc.dram_tensor("WBLOB", [128, 904], F32, kind="ExternalInput").ap()
    OUT = nc.dram_tensor("OUT", [2, 32, S], F32, kind="ExternalOutput").ap()

    with tile.TileContext(nc) as tc, ExitStack() as ctx:
        const = ctx.enter_context(tc.tile_pool(name="const", bufs=1))
        big = ctx.enter_context(tc.tile_pool(name="big", bufs=1))

        # ---- load inputs: weights blob, then xt split at k-chunk boundaries
        # (separate tiles -> early projections only wait for their own DMA)
        TW = 1536
        blob = const.tile([128, 904], F32R, tag="blob")
        nc.sync.dma_start(blob[:], WBLOB.bitcast(F32R))
        xbounds = [(c * TW, min((c + 1) * TW, S)) for c in range((S + TW - 1) // TW)]
        xts = []
        for ci, (lo, hi) in enumerate(xbounds):
            t = big.tile([128, hi - lo], F32R, tag=f"xt{ci}", name=f"xt{ci}")
            nc.sync.dma_start(t[:], XT[:, lo:hi].bitcast(F32R))
            xts.append(t)

        def xsl(col, w):
            ci = col // TW
            off = col - ci * TW
            return xts[ci][:, off : off + w]
        wqt3 = [blob[:, 96 * h : 96 * (h + 1)] for h in range(2)]
        wkt3p = [
            [blob[:, 192 + 96 * (3 * h + r) : 192 + 96 * (3 * h + r + 1)] for r in range(3)]
            for h in range(2)
        ]
        wva = blob[:, 768:834]
        bvb = blob[:, 834:900].bitcast(F32)
        bq3 = [blob[0:96, 900 + h : 901 + h].bitcast(F32) for h in range(2)]
        bk3 = [blob[0:96, 902 + h : 903 + h].bitcast(F32) for h in range(2)]

        # persistent activation tensors
        qt_rep = [
            big.tile([96, S], F32R, tag=f"qt{h}", name=f"qt{h}") for h in range(2)
        ]
        kt_pack = [
            big.tile([96, NG3 * 128], F32R, tag=f"kt{h}", name=f"kt{h}")
            for h in range(2)
        ]
        kt_tail = (
            [
                big.tile([32 * len(tail), 128], F32R, tag=f"ktt{h}", name=f"ktt{h}")
                for h in range(2)
            ]
            if tail
            else None
        )
        vhat = big.tile([128, NJ * 66], F32R, tag="vhat")

        # ---- projections (emitted in xt-tile arrival order; q first so the
        # otherwise-idle ACT engine starts its Identity+bias copies early) ----
        _pools = {}

        def q_chunk(h, n):
            pq = _pools["psp"].tile([128, 512], F32, tag="ps", name=f"pq{h}_{n}")
            nc.tensor.matmul(
                pq[0:96, :], wqt3[h], xsl(n * 512, 512), start=True, stop=True
            )
            if h == 0:
                # ACT is idle before the first exp; use it for h0's bias-copy
                nc.scalar.activation(
                    qt_rep[h][:, n * 512 : (n + 1) * 512],
                    pq[0:96, :],
                    AF.Identity,
                    bias=bq3[h],
                )
            else:
                # h1 projections run mid-attention; keep them off the ACT
                # exp stream
                nc.vector.tensor_scalar(
                    out=qt_rep[h][:, n * 512 : (n + 1) * 512],
                    in0=pq[0:96, :],
                    scalar1=bq3[h],
                    scalar2=None,
                    op0=ALU.add,
                )

        def k_chunk(h, c):
            gs = list(range(4 * c, min(4 * c + 4, NG3)))
            w = len(gs) * 128
            xg = xts[c][:].rearrange("d (g p) -> d g p", p=128)
            pk = _pools["psp"].tile([128, 512], F32, tag="ps", name=f"pk{h}_{c}")
            for r in range(3):
                # g-strided rhs: j = 3g + r for g in gs (local to tile c)
                rhs = xg[:, r : 3 * len(gs) : 3, :]
                nc.tensor.matmul(
                    pk[0:96, 0:w], wkt3p[h][r], rhs, start=(r == 0), stop=(r == 2)
                )
            nc.vector.tensor_scalar(
                out=kt_pack[h][:, 128 * gs[0] : 128 * gs[0] + w],
                in0=pk[0:96, 0:w],
                scalar1=bk3[h],
                scalar2=None,
                op0=ALU.add,
            )

        def k_tail(h):
            pk = _pools["psp"].tile([128, 512], F32, tag="ps", name=f"pkt{h}")
            nt = len(tail)
            for r in range(nt):
                nc.tensor.matmul(
                    pk[0 : 32 * nt, 0:128],
                    wkt3p[h][r][:, 0 : 32 * nt],
                    xsl(tail[r] * 128, 128),
                    start=(r == 0),
                    stop=(r == nt - 1),
                )
            nc.vector.tensor_scalar(
                out=kt_tail[h][:],
                in0=pk[0 : 32 * nt, 0:128],
                scalar1=bk3[h][0 : 32 * nt],
                scalar2=None,
                op0=ALU.add,
            )

        def v_chunk(j):
            pv = _pools["psp"].tile([128, 512], F32, tag="ps", name=f"pv{j}")
            nc.tensor.matmul(
                pv[:, 0:66], xsl(j * 128, 128), wva, start=True, stop=True
            )
            nc.vector.tensor_tensor(
                out=vhat[:, j * 66 : (j + 1) * 66], in0=pv[:, 0:66], in1=bvb, op=ALU.add
            )

        NCH = len(xts)
        NKC = (NG3 + 3) // 4
        psp_ctx = tc.tile_pool(name="psp", bufs=3, space="PSUM")
        _pools["psp"] = psp_ctx.__enter__()
        for ci in range(NCH):
            lo, hi = xbounds[ci]
            for j in range(lo // 128, hi // 128):
                v_chunk(j)
            if ci < NKC:
                k_chunk(0, ci)
            if tail and ci == NCH - 1:
                k_tail(0)
            for n in range(lo // 512, hi // 512):
                q_chunk(0, n)
        psp_ctx.__exit__(None, None, None)

        # head-1 projections run interleaved with head-0 attention out of a
        # single spare PSUM bank (av is single-buffered below to free it)
        h1_chunks = [lambda c=c: k_chunk(1, c) for c in range(NKC)]
        if tail:
            h1_chunks.append(lambda: k_tail(1))
        h1_chunks += [lambda n=n: q_chunk(1, n) for n in range(S // 512)]

        # preload the ACT exp table before the first real exp (Identity used above)
        scratch = const.tile([1, 1], F32, tag="scr")
        nc.scalar.activation(scratch[:], blob[0:1, 900:901].bitcast(F32), AF.Exp)

        # ---- attention ----
        with (
            tc.tile_pool(name="pss", bufs=2, space="PSUM") as pss,
            tc.tile_pool(name="psav", bufs=1, space="PSUM") as psav,
            tc.tile_pool(name="psp2", bufs=1, space="PSUM") as psp2,
            tc.tile_pool(name="work", bufs=3) as work,
            tc.tile_pool(name="outp", bufs=4) as outp,
        ):
            _pools["psp"] = psp2
            last_g = len(groups) - 1

            def flush(p):
                # exp + PV matmuls for a staged scores group; normalization +
                # output DMA when it completes a block.
                ps, jl, gi, av, h, q0 = p
                w = 512 * len(jl)
                pt = work.tile([128, 1536], F32R, tag="pt", name=f"pt{h}_{q0}_{gi}")
                nc.scalar.activation(pt[:, 0:w], ps[:, 0:w], AF.Exp)
                for r, j in enumerate(jl):
                    nc.tensor.matmul(
                        av[:],
                        vhat[:, j * 66 + h * 33 : j * 66 + h * 33 + 33],
                        pt[:, 512 * r : 512 * (r + 1)],
                        start=(gi == 0 and r == 0),
                        stop=(gi == last_g and r == len(jl) - 1),
                    )
                if gi == last_g:
                    # evacuate av to SBUF (frees the single psum bank), then
                    # normalize out = av_sb[0:32] / av_sb[32] in two column
                    # halves so recip/bcast/mul pipeline across engines
                    av_sb = outp.tile([33, 512], F32, tag="avsb", name=f"as{h}_{q0}")
                    nc.vector.tensor_copy(av_sb[:], av[:])
                    osb = outp.tile([32, 512], F32, tag="osb", name=f"ob{h}_{q0}")
                    for z in range(2):
                        cs = slice(256 * z, 256 * (z + 1))
                        rcp = outp.tile([1, 256], F32, tag=f"rcp{z}", name=f"rc{h}_{q0}_{z}")
                        nc.vector.reciprocal(rcp[:], av_sb[32:33, cs])
                        bc = outp.tile([32, 256], F32, tag=f"bc{z}", name=f"bc{h}_{q0}_{z}")
                        nc.gpsimd.partition_broadcast(bc[:], rcp[:])
                        nc.vector.tensor_mul(osb[:, cs], av_sb[0:32, cs], bc[:])
                    nc.sync.dma_start(OUT[h][:, q0 : q0 + 512], osb[:])

            # software-pipelined emission: each group's scores matmuls are
            # issued one group ahead of the previous group's exp+PV, across
            # block boundaries, so ACT never waits on a fresh scores tile.
            pend = None
            h1_iter = iter(h1_chunks)
            for h in range(2):
                for i0 in range(NQB):
                    if h == 0 and i0 >= 1:
                        # two head-1 projection chunks per early block boundary
                        for _ in range(2):
                            fn = next(h1_iter, None)
                            if fn is not None:
                                fn()
                    elif h == 1 and i0 == 0:
                        for fn in h1_iter:  # drain any not yet emitted
                            fn()
                    q0 = i0 * 512
                    av = psav.tile([33, 512], F32, tag="av", name=f"av{h}_{q0}")
                    for gi, jl in enumerate(groups):
                        ps = pss.tile([128, 1536], F32, tag="s", name=f"s{h}_{q0}_{gi}")
                        for r, j in enumerate(jl):
                            if gi < NG3:
                                lhsT = kt_pack[h][32 * r : 32 * r + 32, gi * 128 : (gi + 1) * 128]
                            else:
                                lhsT = kt_tail[h][32 * r : 32 * r + 32, :]
                            nc.tensor.matmul(
                                ps[:, 512 * r : 512 * (r + 1)],
                                lhsT,
                                qt_rep[h][32 * r : 32 * r + 32, q0 : q0 + 512],
                                start=True,
                                stop=True,
                                tile_position=(32 * r, 0),
                            )
                        if pend is not None:
                            flush(pend)
                        pend = (ps, jl, gi, av, h, q0)
            flush(pend)

    nc.compile()
    return nc


def _host_prep(x, Wq, bq, Wk, bk, Wv, bv, S):
    """Per-core input maps."""
    in_maps = []
    for c in range(NCORES):
        b, hp = c // 2, c % 2
        h0, h1 = 2 * hp, 2 * hp + 1
        xt = np.ascontiguousarray(x[b].T).astype(np.float32)  # [128, S]
        blob = np.zeros((128, 904), np.float32)
        for i, hh in enumerate((h0, h1)):
            wq_h = Wq[hh * 32 : (hh + 1) * 32, :]  # [32, 128]
            wk_h = Wk[hh * 32 : (hh + 1) * 32, :]
            blob[:, 96 * i : 96 * (i + 1)] = np.tile(wq_h.T, (1, 3))
            for r in range(3):
                blob[:, 192 + 96 * (3 * i + r) + 32 * r : 192 + 96 * (3 * i + r) + 32 * r + 32] = wk_h.T
            blob[:, 768 + 33 * i : 768 + 33 * i + 32] = Wv[hh * 32 : (hh + 1) * 32, :].T
            blob[:, 834 + 33 * i : 834 + 33 * i + 32] = bv[hh * 32 : (hh + 1) * 32][None, :]
            blob[:, 834 + 33 * i + 32] = 1.0
            blob[0:96, 900 + i] = np.tile(bq[hh * 32 : (hh + 1) * 32], 3)
            blob[0:96, 902 + i] = np.tile(bk[hh * 32 : (hh + 1) * 32], 3)
        in_maps.append({"XT": xt, "WBLOB": blob})
    return in_maps


def _unshard(results, S):
    out = np.empty((B, S, D), np.float32)
    for c in range(NCORES):
        b, hp = c // 2, c % 2
        oc = results[c]["OUT"]  # [2, 32, S]
        for hl in range(2):
            hh = 2 * hp + hl
            out[b, :, hh * 32 : (hh + 1) * 32] = oc[hl].T
    return out


def _run_once(args):
    x, Wq, bq, Wk, bk, Wv, bv = args
    S = x.shape[1]
    if S not in _built:
        _built[S] = build_nc(S)
    nc = _built[S]
    in_maps = _host_prep(x, Wq, bq, Wk, bk, Wv, bv, S)
    res = bass_utils.run_bass_kernel_spmd(nc, in_maps, core_ids=list(range(NCORES)))
    return _unshard(res.results, S)


def _subproc_entry(args):
    return _run_once(args)


def kernel(x, Wq, bq, Wk, bk, Wv, bv):
    args = tuple(
        np.asarray(a, dtype=np.float32) for a in (x, Wq, bq, Wk, bk, Wv, bv)
    )
    # The axon/NRT stack occasionally fails a first dispatch with
    # NRT_EXEC_UNIT_UNRECOVERABLE (device auto-recovers). Retry in-process,
    # then in a fresh spawned process (compile caches make that cheap).
    try:
        return _run_once(args)
    except Exception:
        try:
            return _run_once(args)
        except Exception:
            import multiprocessing as mp

            ctx = mp.get_context("spawn")
            with ctx.Pool(1) as pool:
                return pool.apply(_subproc_entry, (args,))



# revision 6
# speedup vs baseline: 1.4929x; 1.4929x over previous
"""Multi-head self-attention (B=4, S=4096, D=128, H=4, no scaling, no mask)
on 8 Trainium2 NeuronCores.

Sharding: 16 (batch, head) pairs over 8 cores -> core c handles batch c//2,
heads 2*(c%2) and 2*(c%2)+1. No cross-core communication.

Per-core algorithm (flash-style, scores never touch DRAM):
  xT [128, S] in SBUF; q projection pre-scaled by A = 128*log2(e) and
  pre-biased (k bias dropped -- it is softmax-invariant), so the scores
  matmul emits t = A*s directly.
  kt_pack[h] [64, S/2*...]: k chunks packed 2-per-64-partitions; qt_rep[h]
  [64, S] holds q replicated on both 32-partition bands.
  per (head h, q-block of 512): 16 groups of 2 k-chunks:
    2 row-tiled matmuls (tile_position=(32r,0)) -> scoresT psum [128, 1024]
    exp: alternating engines --
      ACT:  pt_bf16 = Exp(t / A)                      (table exp)
      DVE:  pt_i16  = int16(t + B16)  bitcast bf16    (Schraudolph exp:
            the int16 bits ARE the bf16 of e^s up to ~4% sawtooth error)
    PV transposed, bf16: av[q=128, 33] += pt_chunk.T @ vhat_j  (N=33/matmul;
    vhat has a ones column so av[:,32] is the softmax denominator)
  normalization: ACT copies av psum->SBUF, DVE reciprocal_approx_fast,
  GpSimd per-partition-scalar multiply, DMA out [q, 32] slabs.
Host gathers OUT [2, S, 32] per core into the full (B, S, D) output.
"""

import sys

for _p in ("/opt/trn_rl_repo", "/root/.axon_site/_ro/trn_rl_repo"):
    if _p not in sys.path:
        sys.path.append(_p)

import numpy as np
from collections import deque
from contextlib import ExitStack

import concourse.bass as bass
import concourse.bacc as bacc
import concourse.mybir as mybir
import concourse.tile as tile
from concourse import bass_utils

F32 = mybir.dt.float32
F32R = mybir.dt.float32r
BF16 = mybir.dt.bfloat16
I16 = mybir.dt.int16
AF = mybir.ActivationFunctionType
ALU = mybir.AluOpType

B, D, H, HD = 4, 128, 4, 32
NCORES = 8

import os
_ABL_ALL_ACT = os.environ.get("ABL_ALL_ACT", "0") == "1"
_ABL_DVE_NORM = os.environ.get("ABL_DVE_NORM", "0") == "1"

A_SCALE = float(np.float32(128.0 / np.log(2.0)))  # t = A*s
SCALE_INV = float(np.float32(np.log(2.0) / 128.0))  # ACT: exp(t*SCALE_INV)
B16C = float(np.float32(127.0 * 128.0 - 4.456))  # Schraudolph bf16 bias

_built = {}


def build_nc(S):
    """Build + compile the per-core program (identical across cores)."""
    NJ = S // 128  # 128-wide k-chunks
    NQB = S // 512  # 512-wide q-blocks
    NG = NJ // 2  # 2-chunk score groups per (h, q-block)
    NKC = NJ // 8  # k-projection psum tiles per head (8 chunks each)
    TW = 1024
    NCH = S // TW

    nc = bacc.Bacc("TRN2", target_bir_lowering=False, debug=False)

    XT = nc.dram_tensor("XT", [128, S], F32, kind="ExternalInput").ap()
    WBLOB = nc.dram_tensor("WBLOB", [128, 518], F32, kind="ExternalInput").ap()
    OUT = nc.dram_tensor("OUT", [2, S, 32], F32, kind="ExternalOutput").ap()

    with tile.TileContext(nc) as tc, ExitStack() as ctx:
        const = ctx.enter_context(tc.tile_pool(name="const", bufs=1))
        big = ctx.enter_context(tc.tile_pool(name="big", bufs=1))

        blob = const.tile([128, 518], F32R, tag="blob")
        nc.sync.dma_start(blob[:], WBLOB.bitcast(F32R))
        xts = []
        for ci in range(NCH):
            t = big.tile([128, TW], F32R, tag=f"xt{ci}", name=f"xt{ci}")
            nc.sync.dma_start(t[:], XT[:, ci * TW : (ci + 1) * TW].bitcast(F32R))
            xts.append(t)

        wqt2 = [blob[:, 64 * h : 64 * (h + 1)] for h in range(2)]
        wk2p = [
            [blob[:, 128 + 64 * (2 * h + r) : 128 + 64 * (2 * h + r) + 64] for r in range(2)]
            for h in range(2)
        ]
        wva = blob[:, 384:450]
        bvb = blob[:, 450:516].bitcast(F32)
        bq2 = [blob[0:64, 516 + h : 517 + h].bitcast(F32) for h in range(2)]

        qt_rep = [big.tile([64, S], F32R, tag=f"qt{h}", name=f"qt{h}") for h in range(2)]
        kt_pack = [
            big.tile([64, NKC * 512], F32R, tag=f"kt{h}", name=f"kt{h}") for h in range(2)
        ]
        vhat = big.tile([128, NJ * 66], BF16, tag="vhat")

        # ---- projections ----
        _pools = {}

        def v_chunk(j):
            ci, off = j // 8, 128 * (j % 8)
            pv = _pools["psp"].tile([128, 512], F32, tag="ps", name=f"pv{j}")
            nc.tensor.matmul(
                pv[:, 0:66], xts[ci][:, off : off + 128], wva, start=True, stop=True
            )
            nc.vector.tensor_tensor(
                out=vhat[:, j * 66 : (j + 1) * 66], in0=pv[:, 0:66], in1=bvb, op=ALU.add
            )

        def k_group(h, c):
            pk = _pools["psp"].tile([128, 512], F32, tag="ps", name=f"pk{h}_{c}")
            xg = xts[c][:].rearrange("d (g p) -> d g p", p=128)
            for r in range(2):
                rhs = xg[:, r : 8 : 2, :]  # chunks j = 8c + 2g + r, g=0..3
                nc.tensor.matmul(
                    pk[0:64, 0:512], wk2p[h][r], rhs, start=(r == 0), stop=(r == 1)
                )
            dst = kt_pack[h][:, 512 * c : 512 * (c + 1)]
            if h == 0:
                nc.scalar.copy(dst, pk[0:64, 0:512])
            else:
                nc.vector.tensor_copy(dst, pk[0:64, 0:512])

        def q_chunk(h, n):
            ci, off = n // 2, 512 * (n % 2)
            pq = _pools["psp"].tile([128, 512], F32, tag="ps", name=f"pq{h}_{n}")
            nc.tensor.matmul(
                pq[0:64, :], wqt2[h], xts[ci][:, off : off + 512], start=True, stop=True
            )
            dst = qt_rep[h][:, n * 512 : (n + 1) * 512]
            if h == 0:
                nc.scalar.activation(dst, pq[0:64, :], AF.Identity, bias=bq2[h])
            else:
                nc.vector.tensor_scalar(
                    out=dst, in0=pq[0:64, :], scalar1=bq2[h], scalar2=None, op0=ALU.add
                )

        psp_ctx = tc.tile_pool(name="psp", bufs=3, space="PSUM")
        _pools["psp"] = psp_ctx.__enter__()
        for ci in range(NCH):
            for j in range(8 * ci, 8 * ci + 8):
                v_chunk(j)
            k_group(0, ci)
            q_chunk(0, 2 * ci)
            q_chunk(0, 2 * ci + 1)
        for ci in range(NCH):
            k_group(1, ci)
            q_chunk(1, 2 * ci)
            q_chunk(1, 2 * ci + 1)
        psp_ctx.__exit__(None, None, None)

        # preload the ACT exp table before the pipelined exps
        scratch = const.tile([1, 1], F32, tag="scr")
        nc.scalar.activation(scratch[:], blob[0:1, 516:517].bitcast(F32), AF.Exp,
                             scale=SCALE_INV)

        # ---- attention ----
        # PSUM accumulation windows cannot interleave within a bank, so the
        # four per-u PV chains run sequentially against one av bank; each
        # q-block's 16 exp tiles are retained and the PV chains burst
        # interleaved into the NEXT q-block's score loop.
        with (
            tc.tile_pool(name="pss", bufs=3, space="PSUM") as pss,
            tc.tile_pool(name="psav", bufs=2, space="PSUM") as psav,
            tc.tile_pool(name="work", bufs=24) as work,
            tc.tile_pool(name="nrm", bufs=3) as nrm,
        ):
            def emit_norm(av, h, q0):
                avsb = nrm.tile([128, 132], F32, tag="avsb", name=f"as{h}_{q0}")
                nc.scalar.copy(avsb[:], av[:])
                rcp = nrm.tile([128, 132], F32, tag="rcp", name=f"rc{h}_{q0}")
                nc.vector.reciprocal_approx_fast(rcp[:], avsb[:])
                osb = nrm.tile([128, 128], F32, tag="osb", name=f"ob{h}_{q0}")
                for u in range(4):
                    _norm_eng = nc.vector if _ABL_DVE_NORM else nc.gpsimd
                    _norm_eng.tensor_scalar(
                        out=osb[:, 32 * u : 32 * u + 32],
                        in0=avsb[:, 33 * u : 33 * u + 32],
                        scalar1=rcp[:, 33 * u + 32 : 33 * u + 33],
                        scalar2=None,
                        op0=ALU.mult,
                    )
                    nc.sync.dma_start(
                        OUT[h][q0 + 128 * u : q0 + 128 * u + 128, :],
                        osb[:, 32 * u : 32 * u + 32],
                    )

            def make_u_chain(u, pts, av, h, q0):
                def fn():
                    for g in range(NG):
                        for r in range(2):
                            j = 2 * g + r
                            nc.tensor.matmul(
                                av[:, 33 * u : 33 * u + 33],
                                pts[g][:, 512 * r + 128 * u : 512 * r + 128 * u + 128].bitcast(BF16),
                                vhat[:, 66 * j + 33 * h : 66 * j + 33 * h + 33],
                                start=(g == 0 and r == 0),
                                stop=(g == NG - 1 and r == 1),
                            )
                    if u == 3:
                        emit_norm(av, h, q0)

                return fn

            pending_pv = []
            for h in range(2):
                for i0 in range(NQB):
                    q0 = 512 * i0
                    av = psav.tile([128, 132], F32, tag="av", name=f"av{h}_{q0}")
                    pts = []
                    for g in range(NG):
                        ps = pss.tile([128, 1024], F32, tag="s", name=f"s{h}_{q0}_{g}")
                        for r in range(2):
                            nc.tensor.matmul(
                                ps[:, 512 * r : 512 * (r + 1)],
                                kt_pack[h][32 * r : 32 * r + 32, 128 * g : 128 * (g + 1)],
                                qt_rep[h][32 * r : 32 * r + 32, q0 : q0 + 512],
                                start=True,
                                stop=True,
                                tile_position=(32 * r, 0),
                            )
                        # alternate exp engines; ACT gets an extra group on
                        # even q-blocks (8.5/7.5 average split)
                        eng = (g % 2) if not (i0 % 2 == 0 and g == NG - 1) else 0
                        if _ABL_ALL_ACT:
                            eng = 0
                        pt = work.tile([128, 1024], I16, tag="pt", name=f"pt{h}_{q0}_{g}")
                        if eng == 0:
                            nc.scalar.activation(
                                pt[:].bitcast(BF16), ps[:], AF.Exp, scale=SCALE_INV
                            )
                        else:
                            nc.vector.tensor_scalar(
                                out=pt[:], in0=ps[:], scalar1=B16C, scalar2=None,
                                op0=ALU.add,
                            )
                        pts.append(pt)
                        if pending_pv and g % 4 == 1:
                            pending_pv.pop(0)()
                    pending_pv = [make_u_chain(u, pts, av, h, q0) for u in range(4)]
            for fn in pending_pv:
                fn()

    nc.compile()
    return nc


def _host_prep(x, Wq, bq, Wk, bk, Wv, bv, S):
    """Per-core input maps."""
    in_maps = []
    for c in range(NCORES):
        b, hp = c // 2, c % 2
        h0 = 2 * hp
        xt = np.ascontiguousarray(x[b].T).astype(np.float32)  # [128, S]
        blob = np.zeros((128, 518), np.float32)
        for i in range(2):
            hh = h0 + i
            wq_h = Wq[hh * 32 : (hh + 1) * 32, :] * np.float32(A_SCALE)  # [32, 128]
            wk_h = Wk[hh * 32 : (hh + 1) * 32, :]
            blob[:, 64 * i : 64 * (i + 1)] = np.tile(wq_h.T, (1, 2))
            for r in range(2):
                c0 = 128 + 64 * (2 * i + r) + 32 * r
                blob[:, c0 : c0 + 32] = wk_h.T
            blob[:, 384 + 33 * i : 384 + 33 * i + 32] = Wv[hh * 32 : (hh + 1) * 32, :].T
            blob[:, 450 + 33 * i : 450 + 33 * i + 32] = bv[hh * 32 : (hh + 1) * 32][None, :]
            blob[:, 450 + 33 * i + 32] = 1.0
            blob[0:64, 516 + i] = np.tile(bq[hh * 32 : (hh + 1) * 32] * np.float32(A_SCALE), 2)
        in_maps.append({"XT": xt, "WBLOB": blob})
    return in_maps


def _unshard(results, S):
    out = np.empty((B, S, D), np.float32)
    for c in range(NCORES):
        b, hp = c // 2, c % 2
        oc = results[c]["OUT"]  # [2, S, 32]
        for hl in range(2):
            hh = 2 * hp + hl
            out[b, :, hh * 32 : (hh + 1) * 32] = oc[hl]
    return out


def _run_once(args):
    x, Wq, bq, Wk, bk, Wv, bv = args
    S = x.shape[1]
    if S not in _built:
        _built[S] = build_nc(S)
    nc = _built[S]
    in_maps = _host_prep(x, Wq, bq, Wk, bk, Wv, bv, S)
    res = bass_utils.run_bass_kernel_spmd(nc, in_maps, core_ids=list(range(NCORES)))
    return _unshard(res.results, S)


def _subproc_entry(args):
    return _run_once(args)


def kernel(x, Wq, bq, Wk, bk, Wv, bv):
    args = tuple(
        np.asarray(a, dtype=np.float32) for a in (x, Wq, bq, Wk, bk, Wv, bv)
    )
    # The axon/NRT stack occasionally fails a first dispatch with
    # NRT_EXEC_UNIT_UNRECOVERABLE (device auto-recovers). Retry in-process,
    # then in a fresh spawned process (compile caches make that cheap).
    try:
        return _run_once(args)
    except Exception:
        try:
            return _run_once(args)
        except Exception:
            import multiprocessing as mp

            ctx = mp.get_context("spawn")
            with ctx.Pool(1) as pool:
                return pool.apply(_subproc_entry, (args,))


# revision 10
# speedup vs baseline: 1.5297x; 1.0246x over previous
"""Multi-head self-attention (B=4, S=4096, D=128, H=4, no scaling, no mask)
on 8 Trainium2 NeuronCores.

Sharding: 16 (batch, head) pairs over 8 cores -> core c handles batch c//2,
heads 2*(c%2) and 2*(c%2)+1. No cross-core communication.

Per-core algorithm (flash-style, scores never touch DRAM):
  The k-projection is folded into the scores matmul:
      s_eff[k, q] = (q_q + bq) . k_k = x_k^T (Wk^T Wq x_q + Wk^T bq)
  so with y = A*(Wk^T Wq x + Wk^T bq) precomputed per head (A = 128*log2 e),
  scoresT t = A*s comes from matmul(lhsT=x_chunk[128d,128k], rhs=y[:,q512]).
  The k-bias is dropped entirely (softmax-invariant).
  exp, alternating engines per 2-chunk group:
      ACT:  pt_bf16 = Exp(t / A)                    (table exp)
      DVE:  pt_i16  = int16(t + B16) bitcast bf16   (Schraudolph: the int16
            bits ARE the bf16 of e^s up to ~4% sawtooth error)
  PV transposed in bf16: av[q=128, 33] += pt_chunk^T @ vhat_j (N=33/matmul;
  vhat carries a ones column so av[:,32] is the softmax denominator).
  PSUM accumulation windows cannot interleave within a bank, so the four
  per-u PV chains run sequentially against one av bank; each q-block's 16
  exp tiles are retained and the PV chains burst interleaved into the next
  q-block's score loop.
  normalization: ACT copies av psum->SBUF, DVE reciprocal_approx_fast on the
  four denominator columns, GpSimd per-partition-scalar multiply, DMA out.
Host gathers OUT [2, S, 32] per core into the full (B, S, D) output.
"""

import sys

for _p in ("/opt/trn_rl_repo", "/root/.axon_site/_ro/trn_rl_repo"):
    if _p not in sys.path:
        sys.path.append(_p)

import os
import numpy as np
from contextlib import ExitStack

import concourse.bass as bass
import concourse.bacc as bacc
import concourse.mybir as mybir
import concourse.tile as tile
from concourse import bass_utils

F32 = mybir.dt.float32
F32R = mybir.dt.float32r
BF16 = mybir.dt.bfloat16
I16 = mybir.dt.int16
AF = mybir.ActivationFunctionType
ALU = mybir.AluOpType

B, D, H, HD = 4, 128, 4, 32
NCORES = 8

A_SCALE = float(np.float32(128.0 / np.log(2.0)))  # t = A*s
SCALE_INV = float(np.float32(np.log(2.0) / 128.0))  # ACT: exp(t*SCALE_INV)
B16C = float(np.float32(127.0 * 128.0 - 4.456))  # Schraudolph bf16 bias

_built = {}


def build_nc(S):
    """Build + compile the per-core program (identical across cores)."""
    NJ = S // 128  # 128-wide k-chunks
    NQB = S // 512  # 512-wide q-blocks
    NG = NJ // 2  # 2-chunk score groups per (h, q-block)
    TW = 1024
    NCH = S // TW

    nc = bacc.Bacc("TRN2", target_bir_lowering=False, debug=False)

    XT = nc.dram_tensor("XT", [128, S], F32, kind="ExternalInput").ap()
    WBLOB = nc.dram_tensor("WBLOB", [128, 456], F32, kind="ExternalInput").ap()
    OUT = nc.dram_tensor("OUT", [2, S, 32], F32, kind="ExternalOutput").ap()

    with tile.TileContext(nc) as tc, ExitStack() as ctx:
        const = ctx.enter_context(tc.tile_pool(name="const", bufs=1))
        big = ctx.enter_context(tc.tile_pool(name="big", bufs=1))

        blob = const.tile([128, 456], F32R, tag="blob")
        nc.sync.dma_start(blob[:], WBLOB.bitcast(F32R))
        xts = []
        for ci in range(NCH):
            t = big.tile([128, TW], F32R, tag=f"xt{ci}", name=f"xt{ci}")
            nc.sync.dma_start(t[:], XT[:, ci * TW : (ci + 1) * TW].bitcast(F32R))
            xts.append(t)

        def xsl(col, w):
            ci = col // TW
            return xts[ci][:, col - ci * TW : col - ci * TW + w]

        mT = [blob[:, 128 * h : 128 * (h + 1)] for h in range(2)]
        wva = blob[:, 256:322]
        bvb2 = blob[:, 322:454].bitcast(F32)
        ybias = [blob[:, 454 + h : 455 + h].bitcast(F32) for h in range(2)]

        yh = [big.tile([128, S], F32R, tag=f"y{h}", name=f"y{h}") for h in range(2)]
        vhat = big.tile([128, NJ * 66], BF16, tag="vhat")

        with (
            tc.tile_pool(name="pss", bufs=3, space="PSUM") as pss,
            tc.tile_pool(name="psav", bufs=2, space="PSUM") as psav,
            tc.tile_pool(name="work", bufs=24) as work,
            tc.tile_pool(name="nrm", bufs=3) as nrm,
        ):
            def v_pair(j):
                # chunks j, j+1 into one psum tile (two complete matmul
                # windows), one fused bias-add+bf16 copy
                pv = pss.tile([128, 1024], F32, tag="s", name=f"pv{j}")
                for t in range(2):
                    nc.tensor.matmul(
                        pv[:, 66 * t : 66 * t + 66], xsl((j + t) * 128, 128), wva,
                        start=True, stop=True,
                    )
                nc.vector.tensor_tensor(
                    out=vhat[:, j * 66 : (j + 2) * 66], in0=pv[:, 0:132], in1=bvb2,
                    op=ALU.add,
                )

            def y_chunk(h, n):
                py = pss.tile([128, 1024], F32, tag="s", name=f"py{h}_{n}")
                nc.tensor.matmul(
                    py[:, 0:512], mT[h], xsl(n * 512, 512), start=True, stop=True
                )
                dst = yh[h][:, n * 512 : (n + 1) * 512]
                if h == 0:
                    nc.scalar.activation(dst, py[:, 0:512], AF.Identity, bias=ybias[h])
                else:
                    nc.vector.tensor_scalar(
                        out=dst, in0=py[:, 0:512], scalar1=ybias[h], scalar2=None,
                        op0=ALU.add,
                    )

            def emit_norm(av, h, q0):
                avsb = nrm.tile([128, 132], F32, tag="avsb", name=f"as{h}_{q0}")
                nc.scalar.copy(avsb[:], av[:])
                rcp = nrm.tile([128, 4], F32, tag="rcp", name=f"rc{h}_{q0}")
                dens = avsb[:].rearrange("p (u c) -> p u c", c=33)[:, :, 32:33]
                nc.vector.reciprocal_approx_fast(rcp[:], dens)
                osb = nrm.tile([128, 128], F32, tag="osb", name=f"ob{h}_{q0}")
                for u in range(4):
                    nc.gpsimd.tensor_scalar(
                        out=osb[:, 32 * u : 32 * u + 32],
                        in0=avsb[:, 33 * u : 33 * u + 32],
                        scalar1=rcp[:, u : u + 1],
                        scalar2=None,
                        op0=ALU.mult,
                    )
                    nc.sync.dma_start(
                        OUT[h][q0 + 128 * u : q0 + 128 * u + 128, :],
                        osb[:, 32 * u : 32 * u + 32],
                    )

            def make_u_chain(u, pts, av, h, q0):
                def fn():
                    for g in range(NG):
                        for r in range(2):
                            j = 2 * g + r
                            nc.tensor.matmul(
                                av[:, 33 * u : 33 * u + 33],
                                pts[g][:, 512 * r + 128 * u : 512 * r + 128 * u + 128].bitcast(BF16),
                                vhat[:, 66 * j + 33 * h : 66 * j + 33 * h + 33],
                                start=(g == 0 and r == 0),
                                stop=(g == NG - 1 and r == 1),
                            )
                    if u == 3:
                        emit_norm(av, h, q0)

                return fn

            # exp-table preload before the first pipelined Exp
            scratch = const.tile([1, 1], F32, tag="scr")
            nc.scalar.activation(scratch[:], blob[0:1, 454:455].bitcast(F32), AF.Exp,
                                 scale=SCALE_INV)

            pending_pv = []
            for h in range(2):
                for i0 in range(NQB):
                    q0 = 512 * i0
                    if h == 0 and i0 == 0:
                        y_chunk(0, 0)
                    av = psav.tile([128, 132], F32, tag="av", name=f"av{h}_{q0}")
                    pts = []
                    for g in range(NG):
                        if h == 0 and i0 == 0:
                            # fold the v projection into the first q-block
                            v_pair(2 * g)
                        ps = pss.tile([128, 1024], F32, tag="s", name=f"s{h}_{q0}_{g}")
                        for r in range(2):
                            nc.tensor.matmul(
                                ps[:, 512 * r : 512 * (r + 1)],
                                xsl((2 * g + r) * 128, 128),
                                yh[h][:, q0 : q0 + 512],
                                start=True,
                                stop=True,
                            )
                        # alternate exp engines; ACT gets an extra group on
                        # even q-blocks (8.5/7.5 average split)
                        eng = (g % 2) if not (i0 % 2 == 0 and g == NG - 1) else 0
                        pt = work.tile([128, 1024], I16, tag="pt", name=f"pt{h}_{q0}_{g}")
                        if eng == 0:
                            nc.scalar.activation(
                                pt[:].bitcast(BF16), ps[:], AF.Exp, scale=SCALE_INV
                            )
                        else:
                            nc.vector.tensor_scalar(
                                out=pt[:], in0=ps[:], scalar1=B16C, scalar2=None,
                                op0=ALU.add,
                            )
                        pts.append(pt)
                        if pending_pv and g % 4 == 1:
                            pending_pv.pop(0)()
                        # stage upcoming y projections mid-stream
                        if g == 8:
                            if h == 0 and i0 < NQB - 1:
                                y_chunk(0, i0 + 1)
                            elif h == 0 and i0 == NQB - 1:
                                y_chunk(1, 0)
                            elif h == 1 and i0 < NQB - 1:
                                y_chunk(1, i0 + 1)
                    pending_pv = [make_u_chain(u, pts, av, h, q0) for u in range(4)]
            for fn in pending_pv:
                fn()

    nc.compile()
    return nc


def _host_prep(x, Wq, bq, Wk, bk, Wv, bv, S):
    """Per-core input maps."""
    in_maps = []
    for c in range(NCORES):
        b, hp = c // 2, c % 2
        h0 = 2 * hp
        xt = np.ascontiguousarray(x[b].T).astype(np.float32)  # [128, S]
        blob = np.zeros((128, 456), np.float32)
        for i in range(2):
            hh = h0 + i
            wq_h = Wq[hh * 32 : (hh + 1) * 32, :].astype(np.float64)
            wk_h = Wk[hh * 32 : (hh + 1) * 32, :].astype(np.float64)
            bq_h = bq[hh * 32 : (hh + 1) * 32].astype(np.float64)
            # y = A*(Wk^T Wq x + Wk^T bq); lhsT for y-proj is (Wk^T Wq)^T = Wq^T Wk
            blob[:, 128 * i : 128 * (i + 1)] = (A_SCALE * (wq_h.T @ wk_h)).astype(np.float32)
            blob[:, 454 + i] = (A_SCALE * (wk_h.T @ bq_h)).astype(np.float32)
            blob[:, 256 + 33 * i : 256 + 33 * i + 32] = Wv[hh * 32 : (hh + 1) * 32, :].T
            for rep in range(2):
                c0 = 322 + 66 * rep + 33 * i
                blob[:, c0 : c0 + 32] = bv[hh * 32 : (hh + 1) * 32][None, :]
                blob[:, c0 + 32] = 1.0
        in_maps.append({"XT": xt, "WBLOB": blob})
    return in_maps


def _unshard(results, S):
    out = np.empty((B, S, D), np.float32)
    for c in range(NCORES):
        b, hp = c // 2, c % 2
        oc = results[c]["OUT"]  # [2, S, 32]
        for hl in range(2):
            hh = 2 * hp + hl
            out[b, :, hh * 32 : (hh + 1) * 32] = oc[hl]
    return out


def _run_once(args):
    x, Wq, bq, Wk, bk, Wv, bv = args
    S = x.shape[1]
    if S not in _built:
        _built[S] = build_nc(S)
    nc = _built[S]
    in_maps = _host_prep(x, Wq, bq, Wk, bk, Wv, bv, S)
    res = bass_utils.run_bass_kernel_spmd(nc, in_maps, core_ids=list(range(NCORES)))
    return _unshard(res.results, S)


def _subproc_entry(args):
    return _run_once(args)


def kernel(x, Wq, bq, Wk, bk, Wv, bv):
    args = tuple(
        np.asarray(a, dtype=np.float32) for a in (x, Wq, bq, Wk, bk, Wv, bv)
    )
    # The axon/NRT stack occasionally fails a first dispatch with
    # NRT_EXEC_UNIT_UNRECOVERABLE (device auto-recovers). Retry in-process,
    # then in a fresh spawned process (compile caches make that cheap).
    try:
        return _run_once(args)
    except Exception:
        try:
            return _run_once(args)
        except Exception:
            import multiprocessing as mp

            ctx = mp.get_context("spawn")
            with ctx.Pool(1) as pool:
                return pool.apply(_subproc_entry, (args,))


# revision 11
# speedup vs baseline: 1.5518x; 1.0144x over previous
"""Multi-head self-attention (B=4, S=4096, D=128, H=4, no scaling, no mask)
on 8 Trainium2 NeuronCores.

Sharding: 16 (batch, head) pairs over 8 cores -> core c handles batch c//2,
heads 2*(c%2) and 2*(c%2)+1. No cross-core communication.

Per-core algorithm (flash-style, scores never touch DRAM):
  The k-projection is folded into the scores matmul:
      s_eff[k, q] = (q_q + bq) . k_k = x_k^T (Wk^T Wq x_q + Wk^T bq)
  so with y = A*(Wk^T Wq x + Wk^T bq) precomputed per head (A = 128*log2 e),
  scoresT t = A*s comes from matmul(lhsT=x_chunk[128d,128k], rhs=y[:,q512]).
  The k-bias is dropped entirely (softmax-invariant).
  exp, alternating engines per 2-chunk group:
      ACT:  pt_bf16 = Exp(t / A)                    (table exp)
      DVE:  pt_i16  = int16(t + B16) bitcast bf16   (Schraudolph: the int16
            bits ARE the bf16 of e^s up to ~4% sawtooth error)
  PV transposed in bf16: av[q=128, 33] += pt_chunk^T @ vhat_j (N=33/matmul;
  vhat carries a ones column so av[:,32] is the softmax denominator).
  PSUM accumulation windows cannot interleave within a bank, so the four
  per-u PV chains run sequentially against one av bank; each q-block's 16
  exp tiles are retained and the PV chains burst interleaved into the next
  q-block's score loop.
  normalization: ACT copies av psum->SBUF, DVE reciprocal_approx_fast on the
  four denominator columns, GpSimd per-partition-scalar multiply, DMA out.
Host gathers OUT [2, S, 32] per core into the full (B, S, D) output.
"""

import sys

for _p in ("/opt/trn_rl_repo", "/root/.axon_site/_ro/trn_rl_repo"):
    if _p not in sys.path:
        sys.path.append(_p)

import os
import numpy as np
from contextlib import ExitStack

import concourse.bass as bass
import concourse.bacc as bacc
import concourse.mybir as mybir
import concourse.tile as tile
from concourse import bass_utils

F32 = mybir.dt.float32
F32R = mybir.dt.float32r
BF16 = mybir.dt.bfloat16
I16 = mybir.dt.int16
AF = mybir.ActivationFunctionType
ALU = mybir.AluOpType

B, D, H, HD = 4, 128, 4, 32
NCORES = 8

A_SCALE = float(np.float32(128.0 / np.log(2.0)))  # t = A*s
SCALE_INV = float(np.float32(np.log(2.0) / 128.0))  # ACT: exp(t*SCALE_INV)
B16C = float(np.float32(127.0 * 128.0 - 4.456))  # Schraudolph bf16 bias

_built = {}


def build_nc(S):
    """Build + compile the per-core program (identical across cores)."""
    NJ = S // 128  # 128-wide k-chunks
    NQB = S // 512  # 512-wide q-blocks
    NG = NJ // 2  # 2-chunk score groups per (h, q-block)
    TW = 512
    NCH = S // TW

    nc = bacc.Bacc("TRN2", target_bir_lowering=False, debug=False)

    XT = nc.dram_tensor("XT", [128, S], F32, kind="ExternalInput").ap()
    WBLOB = nc.dram_tensor("WBLOB", [128, 456], F32, kind="ExternalInput").ap()
    OUT = nc.dram_tensor("OUT", [2, S, 32], F32, kind="ExternalOutput").ap()

    with tile.TileContext(nc) as tc, ExitStack() as ctx:
        const = ctx.enter_context(tc.tile_pool(name="const", bufs=1))
        big = ctx.enter_context(tc.tile_pool(name="big", bufs=1))

        blob = const.tile([128, 456], F32R, tag="blob")
        nc.sync.dma_start(blob[:], WBLOB.bitcast(F32R))
        xts = []
        for ci in range(NCH):
            t = big.tile([128, TW], F32R, tag=f"xt{ci}", name=f"xt{ci}")
            nc.sync.dma_start(t[:], XT[:, ci * TW : (ci + 1) * TW].bitcast(F32R))
            xts.append(t)

        def xsl(col, w):
            ci = col // TW
            return xts[ci][:, col - ci * TW : col - ci * TW + w]

        mT = [blob[:, 128 * h : 128 * (h + 1)] for h in range(2)]
        wva = blob[:, 256:322]
        bvb2 = blob[:, 322:454].bitcast(F32)
        ybias = [blob[:, 454 + h : 455 + h].bitcast(F32) for h in range(2)]

        yh = [big.tile([128, S], F32R, tag=f"y{h}", name=f"y{h}") for h in range(2)]
        vhat = big.tile([128, NJ * 66], BF16, tag="vhat")

        with (
            tc.tile_pool(name="pss", bufs=3, space="PSUM") as pss,
            tc.tile_pool(name="psav", bufs=1, space="PSUM") as psav,
            tc.tile_pool(name="psy", bufs=1, space="PSUM") as psy,
            tc.tile_pool(name="work", bufs=24) as work,
            tc.tile_pool(name="nrm", bufs=3) as nrm,
        ):
            def v_pair(j):
                # chunks j, j+1 into one psum tile (two complete matmul
                # windows), one fused bias-add+bf16 copy
                pv = pss.tile([128, 1024], F32, tag="s", name=f"pv{j}")
                for t in range(2):
                    nc.tensor.matmul(
                        pv[:, 66 * t : 66 * t + 66], xsl((j + t) * 128, 128), wva,
                        start=True, stop=True,
                    )
                nc.vector.tensor_tensor(
                    out=vhat[:, j * 66 : (j + 2) * 66], in0=pv[:, 0:132], in1=bvb2,
                    op=ALU.add,
                )

            def y_chunk(h, n):
                py = psy.tile([128, 512], F32, tag="y", name=f"py{h}_{n}")
                nc.tensor.matmul(
                    py[:], mT[h], xsl(n * 512, 512), start=True, stop=True
                )
                dst = yh[h][:, n * 512 : (n + 1) * 512]
                if h == 0:
                    nc.scalar.activation(dst, py[:], AF.Identity, bias=ybias[h])
                else:
                    nc.vector.tensor_scalar(
                        out=dst, in0=py[:], scalar1=ybias[h], scalar2=None,
                        op0=ALU.add,
                    )

            def emit_norm(av, h, q0):
                avsb = nrm.tile([128, 132], F32, tag="avsb", name=f"as{h}_{q0}")
                nc.scalar.copy(avsb[:], av[:])
                rcp = nrm.tile([128, 4], F32, tag="rcp", name=f"rc{h}_{q0}")
                dens = avsb[:].rearrange("p (u c) -> p u c", c=33)[:, :, 32:33]
                nc.vector.reciprocal_approx_fast(rcp[:], dens)
                osb = nrm.tile([128, 128], F32, tag="osb", name=f"ob{h}_{q0}")
                for u in range(4):
                    nc.gpsimd.tensor_scalar(
                        out=osb[:, 32 * u : 32 * u + 32],
                        in0=avsb[:, 33 * u : 33 * u + 32],
                        scalar1=rcp[:, u : u + 1],
                        scalar2=None,
                        op0=ALU.mult,
                    )
                    nc.sync.dma_start(
                        OUT[h][q0 + 128 * u : q0 + 128 * u + 128, :],
                        osb[:, 32 * u : 32 * u + 32],
                    )

            def make_u_chain(u, pts, av, h, q0, half):
                def fn():
                    for g in range(half * NG // 2, (half + 1) * NG // 2):
                        for r in range(2):
                            j = 2 * g + r
                            nc.tensor.matmul(
                                av[:, 33 * u : 33 * u + 33],
                                pts[g][:, 512 * r + 128 * u : 512 * r + 128 * u + 128].bitcast(BF16),
                                vhat[:, 66 * j + 33 * h : 66 * j + 33 * h + 33],
                                start=(g == 0 and r == 0),
                                stop=(g == NG - 1 and r == 1),
                            )
                    if u == 3 and half == 1:
                        emit_norm(av, h, q0)

                return fn

            # exp-table preload before the first pipelined Exp
            scratch = const.tile([1, 1], F32, tag="scr")
            nc.scalar.activation(scratch[:], blob[0:1, 454:455].bitcast(F32), AF.Exp,
                                 scale=SCALE_INV)

            pending_pv = []
            for h in range(2):
                for i0 in range(NQB):
                    q0 = 512 * i0
                    if h == 0 and i0 == 0:
                        y_chunk(0, 0)
                    av = psav.tile([128, 132], F32, tag="av", name=f"av{h}_{q0}")
                    pts = []
                    for g in range(NG):
                        if h == 0 and i0 == 0:
                            # fold the v projection into the first q-block
                            v_pair(2 * g)
                        ps = pss.tile([128, 1024], F32, tag="s", name=f"s{h}_{q0}_{g}")
                        for r in range(2):
                            nc.tensor.matmul(
                                ps[:, 512 * r : 512 * (r + 1)],
                                xsl((2 * g + r) * 128, 128),
                                yh[h][:, q0 : q0 + 512],
                                start=True,
                                stop=True,
                            )
                        # alternate exp engines; ACT gets an extra group on
                        # even q-blocks (8.5/7.5 average split)
                        eng = (g % 2) if g != NG - 1 else 0
                        pt = work.tile([128, 1024], I16, tag="pt", name=f"pt{h}_{q0}_{g}")
                        if eng == 0:
                            nc.scalar.activation(
                                pt[:].bitcast(BF16), ps[:], AF.Exp, scale=SCALE_INV
                            )
                        else:
                            nc.vector.tensor_scalar(
                                out=pt[:], in0=ps[:], scalar1=B16C, scalar2=None,
                                op0=ALU.add,
                            )
                        pts.append(pt)
                        if pending_pv and g % 2 == 1:
                            pending_pv.pop(0)()
                        # stage upcoming y projections mid-stream
                        if g == 8:
                            if h == 0 and i0 < NQB - 1:
                                y_chunk(0, i0 + 1)
                            elif h == 0 and i0 == NQB - 1:
                                y_chunk(1, 0)
                            elif h == 1 and i0 < NQB - 1:
                                y_chunk(1, i0 + 1)
                    pending_pv = [make_u_chain(u, pts, av, h, q0, hf)
                                  for u in range(4) for hf in range(2)]
            for fn in pending_pv:
                fn()

    nc.compile()
    return nc


def _host_prep(x, Wq, bq, Wk, bk, Wv, bv, S):
    """Per-core input maps."""
    in_maps = []
    for c in range(NCORES):
        b, hp = c // 2, c % 2
        h0 = 2 * hp
        xt = np.ascontiguousarray(x[b].T).astype(np.float32)  # [128, S]
        blob = np.zeros((128, 456), np.float32)
        for i in range(2):
            hh = h0 + i
            wq_h = Wq[hh * 32 : (hh + 1) * 32, :].astype(np.float64)
            wk_h = Wk[hh * 32 : (hh + 1) * 32, :].astype(np.float64)
            bq_h = bq[hh * 32 : (hh + 1) * 32].astype(np.float64)
            # y = A*(Wk^T Wq x + Wk^T bq); lhsT for y-proj is (Wk^T Wq)^T = Wq^T Wk
            blob[:, 128 * i : 128 * (i + 1)] = (A_SCALE * (wq_h.T @ wk_h)).astype(np.float32)
            blob[:, 454 + i] = (A_SCALE * (wk_h.T @ bq_h)).astype(np.float32)
            blob[:, 256 + 33 * i : 256 + 33 * i + 32] = Wv[hh * 32 : (hh + 1) * 32, :].T
            for rep in range(2):
                c0 = 322 + 66 * rep + 33 * i
                blob[:, c0 : c0 + 32] = bv[hh * 32 : (hh + 1) * 32][None, :]
                blob[:, c0 + 32] = 1.0
        in_maps.append({"XT": xt, "WBLOB": blob})
    return in_maps


def _unshard(results, S):
    out = np.empty((B, S, D), np.float32)
    for c in range(NCORES):
        b, hp = c // 2, c % 2
        oc = results[c]["OUT"]  # [2, S, 32]
        for hl in range(2):
            hh = 2 * hp + hl
            out[b, :, hh * 32 : (hh + 1) * 32] = oc[hl]
    return out


def _run_once(args):
    x, Wq, bq, Wk, bk, Wv, bv = args
    S = x.shape[1]
    if S not in _built:
        _built[S] = build_nc(S)
    nc = _built[S]
    in_maps = _host_prep(x, Wq, bq, Wk, bk, Wv, bv, S)
    res = bass_utils.run_bass_kernel_spmd(nc, in_maps, core_ids=list(range(NCORES)))
    return _unshard(res.results, S)


def _subproc_entry(args):
    return _run_once(args)


def kernel(x, Wq, bq, Wk, bk, Wv, bv):
    args = tuple(
        np.asarray(a, dtype=np.float32) for a in (x, Wq, bq, Wk, bk, Wv, bv)
    )
    # The axon/NRT stack occasionally fails a first dispatch with
    # NRT_EXEC_UNIT_UNRECOVERABLE (device auto-recovers). Retry in-process,
    # then in a fresh spawned process (compile caches make that cheap).
    try:
        return _run_once(args)
    except Exception:
        try:
            return _run_once(args)
        except Exception:
            import multiprocessing as mp

            ctx = mp.get_context("spawn")
            with ctx.Pool(1) as pool:
                return pool.apply(_subproc_entry, (args,))


# revision 12
# speedup vs baseline: 1.6066x; 1.0353x over previous
"""Multi-head self-attention (B=4, S=4096, D=128, H=4, no scaling, no mask)
on 8 Trainium2 NeuronCores.

Sharding: 16 (batch, head) pairs over 8 cores -> core c handles batch c//2,
heads 2*(c%2) and 2*(c%2)+1. No cross-core communication.

Per-core algorithm (flash-style, scores never touch DRAM):
  The k-projection is folded into the scores matmul:
      s_eff[k, q] = (q_q + bq) . k_k = x_k^T (Wk^T Wq x_q + Wk^T bq)
  so with y = A*(Wk^T Wq x + Wk^T bq) precomputed per head (A = 128*log2 e),
  scoresT t = A*s comes from matmul(lhsT=x_chunk[128d,128k], rhs=y[:,q512]).
  The k-bias is dropped entirely (softmax-invariant).
  exp, alternating engines per 2-chunk group:
      ACT:  pt_bf16 = Exp(t / A)                    (table exp)
      DVE:  pt_i16  = int16(t + B16) bitcast bf16   (Schraudolph: the int16
            bits ARE the bf16 of e^s up to ~4% sawtooth error)
  PV transposed in bf16: av[q=128, 33] += pt_chunk^T @ vhat_j (N=33/matmul;
  vhat carries a ones column so av[:,32] is the softmax denominator).
  PSUM accumulation windows cannot interleave within a bank, so the four
  per-u PV chains run sequentially against one av bank; each q-block's 16
  exp tiles are retained and the PV chains burst interleaved into the next
  q-block's score loop.
  normalization: ACT copies av psum->SBUF, DVE reciprocal_approx_fast on the
  four denominator columns, GpSimd per-partition-scalar multiply, DMA out.
Host gathers OUT [2, S, 32] per core into the full (B, S, D) output.
"""

import sys

for _p in ("/opt/trn_rl_repo", "/root/.axon_site/_ro/trn_rl_repo"):
    if _p not in sys.path:
        sys.path.append(_p)

import os
import numpy as np
from contextlib import ExitStack

import concourse.bass as bass
import concourse.bacc as bacc
import concourse.mybir as mybir
import concourse.tile as tile
from concourse import bass_utils

F32 = mybir.dt.float32
F32R = mybir.dt.float32r
BF16 = mybir.dt.bfloat16
I16 = mybir.dt.int16
AF = mybir.ActivationFunctionType
ALU = mybir.AluOpType

B, D, H, HD = 4, 128, 4, 32
NCORES = 8

A_SCALE = float(np.float32(128.0 / np.log(2.0)))  # t = A*s
SCALE_INV = float(np.float32(np.log(2.0) / 128.0))  # ACT: exp(t*SCALE_INV)
B16C = float(np.float32(127.0 * 128.0 - 4.456))  # Schraudolph bf16 bias

_built = {}


def build_nc(S):
    """Build + compile the per-core program (identical across cores)."""
    NJ = S // 128  # 128-wide k-chunks
    NQB = S // 512  # 512-wide q-blocks
    NG = NJ // 2  # 2-chunk score groups per (h, q-block)
    TW = 512
    NCH = S // TW

    nc = bacc.Bacc("TRN2", target_bir_lowering=False, debug=False)

    XT = nc.dram_tensor("XT", [128, S], F32, kind="ExternalInput").ap()
    WBLOB = nc.dram_tensor("WBLOB", [128, 456], F32, kind="ExternalInput").ap()
    OUT = nc.dram_tensor("OUT", [2, S, 32], F32, kind="ExternalOutput").ap()

    with tile.TileContext(nc) as tc, ExitStack() as ctx:
        const = ctx.enter_context(tc.tile_pool(name="const", bufs=1))
        big = ctx.enter_context(tc.tile_pool(name="big", bufs=1))

        blob = const.tile([128, 456], F32R, tag="blob")
        nc.sync.dma_start(blob[:], WBLOB.bitcast(F32R))
        xts = []
        for ci in range(NCH):
            t = big.tile([128, TW], F32R, tag=f"xt{ci}", name=f"xt{ci}")
            nc.sync.dma_start(t[:], XT[:, ci * TW : (ci + 1) * TW].bitcast(F32R))
            xts.append(t)

        def xsl(col, w):
            ci = col // TW
            return xts[ci][:, col - ci * TW : col - ci * TW + w]

        mT = [blob[:, 128 * h : 128 * (h + 1)] for h in range(2)]
        wva = blob[:, 256:322]
        bvb2 = blob[:, 322:454].bitcast(F32)
        ybias = [blob[:, 454 + h : 455 + h].bitcast(F32) for h in range(2)]

        yh = [big.tile([128, S], F32R, tag=f"y{h}", name=f"y{h}") for h in range(2)]
        vhat = big.tile([128, NJ * 66], BF16, tag="vhat")

        with (
            tc.tile_pool(name="pss", bufs=3, space="PSUM") as pss,
            tc.tile_pool(name="psav", bufs=1, space="PSUM") as psav,
            tc.tile_pool(name="psy", bufs=1, space="PSUM") as psy,
            tc.tile_pool(name="work", bufs=24) as work,
            tc.tile_pool(name="nrm", bufs=3) as nrm,
        ):
            def v_pair(j):
                # chunks j, j+1 into one psum tile (two complete matmul
                # windows), one fused bias-add+bf16 copy
                pv = pss.tile([128, 1024], F32, tag="s", name=f"pv{j}")
                for t in range(2):
                    nc.tensor.matmul(
                        pv[:, 66 * t : 66 * t + 66], xsl((j + t) * 128, 128), wva,
                        start=True, stop=True,
                    )
                nc.vector.tensor_tensor(
                    out=vhat[:, j * 66 : (j + 2) * 66], in0=pv[:, 0:132], in1=bvb2,
                    op=ALU.add,
                )

            def y_chunk(h, n):
                py = psy.tile([128, 512], F32, tag="y", name=f"py{h}_{n}")
                nc.tensor.matmul(
                    py[:], mT[h], xsl(n * 512, 512), start=True, stop=True
                )
                dst = yh[h][:, n * 512 : (n + 1) * 512]
                if h == 0:
                    nc.scalar.activation(dst, py[:], AF.Identity, bias=ybias[h])
                else:
                    nc.vector.tensor_scalar(
                        out=dst, in0=py[:], scalar1=ybias[h], scalar2=None,
                        op0=ALU.add,
                    )

            def emit_norm(av, h, q0):
                avsb = nrm.tile([128, 132], F32, tag="avsb", name=f"as{h}_{q0}")
                nc.scalar.copy(avsb[:], av[:])
                rcp = nrm.tile([128, 4], F32, tag="rcp", name=f"rc{h}_{q0}")
                dens = avsb[:].rearrange("p (u c) -> p u c", c=33)[:, :, 32:33]
                nc.vector.reciprocal_approx_fast(rcp[:], dens)
                osb = nrm.tile([128, 128], F32, tag="osb", name=f"ob{h}_{q0}")
                for u in range(4):
                    nc.gpsimd.tensor_scalar(
                        out=osb[:, 32 * u : 32 * u + 32],
                        in0=avsb[:, 33 * u : 33 * u + 32],
                        scalar1=rcp[:, u : u + 1],
                        scalar2=None,
                        op0=ALU.mult,
                    )
                    nc.sync.dma_start(
                        OUT[h][q0 + 128 * u : q0 + 128 * u + 128, :],
                        osb[:, 32 * u : 32 * u + 32],
                    )

            def make_u_chain(u, pts, av, h, q0, half):
                def fn():
                    for g in range(half * NG // 2, (half + 1) * NG // 2):
                        for r in range(2):
                            j = 2 * g + r
                            nc.tensor.matmul(
                                av[:, 33 * u : 33 * u + 33],
                                pts[g][:, 512 * r + 128 * u : 512 * r + 128 * u + 128].bitcast(BF16),
                                vhat[:, 66 * j + 33 * h : 66 * j + 33 * h + 33],
                                start=(g == 0 and r == 0),
                                stop=(g == NG - 1 and r == 1),
                            )
                    if u == 3 and half == 1:
                        emit_norm(av, h, q0)

                return fn

            # exp-table preload before the first pipelined Exp
            scratch = const.tile([1, 1], F32, tag="scr")
            nc.scalar.activation(scratch[:], blob[0:1, 454:455].bitcast(F32), AF.Exp,
                                 scale=SCALE_INV)

            pending_pv = []
            v_pair(0)
            v_pair(2)
            y_chunk(0, 0)
            for h in range(2):
                for i0 in range(NQB):
                    q0 = 512 * i0
                    av = psav.tile([128, 132], F32, tag="av", name=f"av{h}_{q0}")
                    pts = []
                    for g in range(NG):
                        if h == 0 and i0 == 0 and g < 12:
                            # spread the v projection through the first q-blocks
                            v_pair(4 + 2 * g)
                        elif h == 0 and i0 == 1 and g < 2:
                            v_pair(28 + 2 * g)
                        ps = pss.tile([128, 1024], F32, tag="s", name=f"s{h}_{q0}_{g}")
                        for r in range(2):
                            nc.tensor.matmul(
                                ps[:, 512 * r : 512 * (r + 1)],
                                xsl((2 * g + r) * 128, 128),
                                yh[h][:, q0 : q0 + 512],
                                start=True,
                                stop=True,
                            )
                        # alternate exp engines; ACT gets an extra group on
                        # even q-blocks (8.5/7.5 average split)
                        eng = (g % 2) if not (g == NG - 1 and i0 % 2 == 0) else 0
                        pt = work.tile([128, 1024], I16, tag="pt", name=f"pt{h}_{q0}_{g}")
                        if eng == 0:
                            nc.scalar.activation(
                                pt[:].bitcast(BF16), ps[:], AF.Exp, scale=SCALE_INV
                            )
                        else:
                            nc.vector.tensor_scalar(
                                out=pt[:], in0=ps[:], scalar1=B16C, scalar2=None,
                                op0=ALU.add,
                            )
                        pts.append(pt)
                        if pending_pv and g % 2 == 1:
                            pending_pv.pop(0)()
                        # stage upcoming y projections mid-stream
                        if g == 14:
                            if h == 0 and i0 < NQB - 1:
                                y_chunk(0, i0 + 1)
                            elif h == 0 and i0 == NQB - 1:
                                y_chunk(1, 0)
                            elif h == 1 and i0 < NQB - 1:
                                y_chunk(1, i0 + 1)
                    pending_pv = [make_u_chain(u, pts, av, h, q0, hf)
                                  for u in range(4) for hf in range(2)]
            for fn in pending_pv:
                fn()

    nc.compile()
    return nc


def _host_prep(x, Wq, bq, Wk, bk, Wv, bv, S):
    """Per-core input maps."""
    in_maps = []
    for c in range(NCORES):
        b, hp = c // 2, c % 2
        h0 = 2 * hp
        xt = np.ascontiguousarray(x[b].T).astype(np.float32)  # [128, S]
        blob = np.zeros((128, 456), np.float32)
        for i in range(2):
            hh = h0 + i
            wq_h = Wq[hh * 32 : (hh + 1) * 32, :].astype(np.float64)
            wk_h = Wk[hh * 32 : (hh + 1) * 32, :].astype(np.float64)
            bq_h = bq[hh * 32 : (hh + 1) * 32].astype(np.float64)
            # y = A*(Wk^T Wq x + Wk^T bq); lhsT for y-proj is (Wk^T Wq)^T = Wq^T Wk
            blob[:, 128 * i : 128 * (i + 1)] = (A_SCALE * (wq_h.T @ wk_h)).astype(np.float32)
            blob[:, 454 + i] = (A_SCALE * (wk_h.T @ bq_h)).astype(np.float32)
            blob[:, 256 + 33 * i : 256 + 33 * i + 32] = Wv[hh * 32 : (hh + 1) * 32, :].T
            for rep in range(2):
                c0 = 322 + 66 * rep + 33 * i
                blob[:, c0 : c0 + 32] = bv[hh * 32 : (hh + 1) * 32][None, :]
                blob[:, c0 + 32] = 1.0
        in_maps.append({"XT": xt, "WBLOB": blob})
    return in_maps


def _unshard(results, S):
    out = np.empty((B, S, D), np.float32)
    for c in range(NCORES):
        b, hp = c // 2, c % 2
        oc = results[c]["OUT"]  # [2, S, 32]
        for hl in range(2):
            hh = 2 * hp + hl
            out[b, :, hh * 32 : (hh + 1) * 32] = oc[hl]
    return out


def _run_once(args):
    x, Wq, bq, Wk, bk, Wv, bv = args
    S = x.shape[1]
    if S not in _built:
        _built[S] = build_nc(S)
    nc = _built[S]
    in_maps = _host_prep(x, Wq, bq, Wk, bk, Wv, bv, S)
    res = bass_utils.run_bass_kernel_spmd(nc, in_maps, core_ids=list(range(NCORES)))
    return _unshard(res.results, S)


def _subproc_entry(args):
    return _run_once(args)


def kernel(x, Wq, bq, Wk, bk, Wv, bv):
    args = tuple(
        np.asarray(a, dtype=np.float32) for a in (x, Wq, bq, Wk, bk, Wv, bv)
    )
    # The axon/NRT stack occasionally fails a first dispatch with
    # NRT_EXEC_UNIT_UNRECOVERABLE (device auto-recovers). Retry in-process,
    # then in a fresh spawned process (compile caches make that cheap).
    try:
        return _run_once(args)
    except Exception:
        try:
            return _run_once(args)
        except Exception:
            import multiprocessing as mp

            ctx = mp.get_context("spawn")
            with ctx.Pool(1) as pool:
                return pool.apply(_subproc_entry, (args,))
